# revision 43
# baseline (speedup 1.0000x reference)
"""GAT-with-gate kernel for Trainium2 (8 NeuronCores), v5.

Row-shards the 8192 receivers across 8 cores (1024 each). The O(N*D^2)
linear algebra is folded on the host; the device runs only the O(N^2*D)
attention core, built around fp8 DoubleRow matmuls (0.5 cyc/row, moving
operand capped at 512 elements by the ISA).

Host folding: e = h asym h^T with h = xW^T + b expands to
  e[i,j] = x_i (W^T asym W) x_j + c1.x_j + c2.x_i + c0 .
The quadratic term is an fp8-DR matmul of e4m3(x^T) (stationary, resident
all of phase 3) against e4m3(Qf^T xloc^T) (qtp, moving). c1.x_j + c0 - 5
ships as an exact f32 per-j row bias (rb); c2.x_i rides in the additive
mask tiles: addm[j,i] = e4m3(c2.x_i) on edges, e4m3(c2.x_i - 40) off
edges, so exp(e + addm - 5) flushes non-edges to exactly 0 in fp8e5m2
(the -5 shift keeps exp in e5m2 range; softmax normalization cancels the
shift and most of the e5m2 quantization noise). haug = e4m3(h + b) rows
[128, 64, 258] ([h+b | 1 | 0]); the ones column accumulates Z.

Phase 3, per j-block, paths strictly alternate so the two psum-drain
engines pipeline against the 3-deep e-psum ring (the steady state is
DVE-paced at 1192 ns/pair):
  P5 (even jb): PE pre-adds addm via identity matmul (bank start=True,
      e-matmuls accumulate), ACT exp(bias=rb) -> pts e5m2
  P3 (odd jb):  DVE STT (e + rb + addm) -> f32 esc, Pool pow(e, esc)
Aggregation: fp8-DR (pts e5m2 stationary, haug e4m3 moving, 2 j-blocks
per matmul; h-part and tiny Z-part split to respect the 512-element
limit). Chains for i-chunks 0-2 stay psum-resident across all 32 pairs,
packed 3-per-2-banks with the Z columns in the spare corner (zero drain
traffic); chunks 3-7 run as tail chains in the freed e-psum banks (all
32 p pair-tiles stay alive in SBUF, 64KB/partition). PSUM: e-ring 6
banks + resident chains 2 banks = all 8.

Phase 4 reads chain psum directly: relu(scale=1/Z), gate dot fused into
one DVE STT with accum_out, sigmoid, blend, DMA out. DMA queues (SP and
Pool) are hand-paced so every mask/x/haug tile lands just before its
first consumer.
"""
import os
import sys

import numpy as np

for _p in ("/opt/trn_rl_repo", "/root/.axon_site/_ro/trn_rl_repo"):
    if os.path.isdir(_p) and _p not in sys.path:
        sys.path.append(_p)

import ml_dtypes  # noqa: E402

import concourse.bass as bass  # noqa: E402
import concourse.mybir as mybir  # noqa: E402
import concourse.tile as tile  # noqa: E402
from concourse import bacc, library_config  # noqa: E402
from concourse.bass_utils import run_bass_kernel_spmd  # noqa: E402

N = 8192
D = 256
M = 8          # cores
NL = N // M    # 1024 local receivers per core
P = 128
JBLK = N // P  # 64 j-blocks
NPAIR = JBLK // 2
ICH = NL // P  # 8 local i-chunks
DA = D + 2     # [h | 1 | 0]

SH = 5.0       # exp shift: p = exp(e - SH); max e ~ 14.7 fits fp8e5m2
BNEG = 40.0    # extra additive mask for non-edges (exp -> 0 in e5m2)

F32 = mybir.dt.float32
BF16 = mybir.dt.bfloat16
FP8 = mybir.dt.float8e4
FP8E5 = mybir.dt.float8e5
AF = mybir.ActivationFunctionType
ALU = mybir.AluOpType
DR = mybir.MatmulPerfMode.DoubleRow

F8NP = ml_dtypes.float8_e4m3
F85NP = ml_dtypes.float8_e5m2

# per-j-block path: 5=PE addm + ACT exp, 3=DVE add + Pool pow.
# Strict-ish alternation keeps the two psum-drain engines pipelined.
PAT16A = [5, 3, 5, 3, 5, 3, 5, 3, 5, 3, 5, 3, 5, 3, 5, 3]
PAT16B = [5, 3, 5, 3, 5, 3, 5, 3, 3, 5, 3, 5, 3, 5, 3, 5]
PATHS = PAT16A * 4

_BUILD_CACHE = {}
LAST_RESULT = None


def _build(paths):
    nc = bacc.Bacc(None, target_bir_lowering=False)

    xthi_d = nc.dram_tensor("xthi", (P, 2, N), FP8, kind="ExternalInput")
    qtp_d = nc.dram_tensor("qtp", (P, 2, NL), FP8, kind="ExternalInput")
    haug_d = nc.dram_tensor("haugd", (P, JBLK, DA), FP8, kind="ExternalInput")
    rb_d = nc.dram_tensor("rb", (P, JBLK), F32, kind="ExternalInput")
    # fpk f32: -gb | -1.0 | gwx(256) | gwh(256) | brow(256)
    fpk_d = nc.dram_tensor("fpk", (P, 2 + 3 * D), F32, kind="ExternalInput")
    idt_d = nc.dram_tensor("idt", (P, P), FP8, kind="ExternalInput")
    mask_d = nc.dram_tensor("mask", (JBLK // 4 * P, 4 * NL), FP8,
                            kind="ExternalInput")
    xloc_d = nc.dram_tensor("xloc", (P, ICH, D), F32, kind="ExternalInput")
    out_d = nc.dram_tensor("out", (P, ICH, D), F32, kind="ExternalOutput")

    with tile.TileContext(nc) as tc:
        with (
            tc.tile_pool(name="const", bufs=1) as cp,
            tc.tile_pool(name="maskp", bufs=8) as maskp,
            tc.tile_pool(name="escp", bufs=4) as escp,
            tc.tile_pool(name="hp4", bufs=6) as hp4,
            tc.tile_pool(name="small", bufs=12) as smallp,
        ):
            nc.gpsimd.load_library(library_config.standard)

            # ---- persistent tiles ----
            fpk = cp.tile([P, 2 + 3 * D], F32, tag="fpk")
            gbt = fpk[:, 0:1]
            gwx_b = fpk[:, 2:2 + D]
            gwh_b = fpk[:, 2 + D:2 + 2 * D]
            brow = fpk[:, 2 + 2 * D:2 + 3 * D]
            idt = cp.tile([P, P], FP8, tag="idt")
            xthi = cp.tile([P, 2, N], FP8, tag="xthi")
            qtp = cp.tile([P, 2, NL], FP8, tag="qtp")
            haug = cp.tile([P, JBLK, DA], FP8, tag="haug")
            rb = cp.tile([P, JBLK], F32, tag="rb")
            ebase = cp.tile([P, NL], BF16, tag="ebase")
            xlp = cp.tile([P, ICH, D], F32, tag="xlp")
            otp = cp.tile([P, ICH, D], F32, tag="otp")
            sxs = [cp.tile([P, 1], F32, tag=f"sx{i}", name=f"sx{i}")
                   for i in range(ICH)]
            pts = [cp.tile([P, 2, NL], FP8E5, tag=f"pt{pb}", name=f"pt{pb}")
                   for pb in range(NPAIR)]

            # ---- phase 1: pure DMA ----
            nc.sync.dma_start(idt[:], idt_d[:])
            mt0 = maskp.tile([P, 4, NL], FP8, tag="mask", name="mask0")
            nc.sync.dma_start(mt0[:, 0:2, :], mask_d[0:P, 0:2 * NL])
            mask_tiles[0] = mt0
            nc.sync.dma_start(xthi[:, :, 0:1024], xthi_d[:, :, 0:1024])
            nc.sync.dma_start(rb[:], rb_d[:])
            nc.sync.dma_start(mt0[:, 2:4, :], mask_d[0:P, 2 * NL:])
            mask_dma(1, nc.sync)
            mask_dma(2, nc.sync)
            nc.sync.dma_start(xthi[:, :, 1024:2048], xthi_d[:, :, 1024:2048])
            mask_dma(3, nc.sync)
            nc.sync.dma_start(xthi[:, :, 2048:4096], xthi_d[:, :, 2048:4096])
            mask_dma(4, nc.sync)
            mask_dma(5, nc.sync)
            nc.sync.dma_start(xthi[:, :, 4096:6144], xthi_d[:, :, 4096:6144])
            mask_dma(6, nc.sync)
            mask_dma(7, nc.sync)
            nc.sync.dma_start(xthi[:, :, 6144:8192], xthi_d[:, :, 6144:8192])

            nc.gpsimd.dma_start(qtp[:], qtp_d[:])
            nc.gpsimd.memset(ebase[:], float(np.e))
            dum = smallp.tile([P, 1], F32, tag="dum")
            nc.gpsimd.memset(dum[:], 0.0)
            dume = smallp.tile([P, 1], F32, tag="dum")
            nc.scalar.activation(dume[:], dum[:], AF.Exp, bias=0.0, scale=1.0)
            nc.gpsimd.dma_start(haug[:, 0:16, :], haug_d[:, 0:16, :])
            nc.gpsimd.dma_start(haug[:, 16:32, :], haug_d[:, 16:32, :])
            nc.gpsimd.dma_start(haug[:, 32:48, :], haug_d[:, 32:48, :])

            # ---- phase 3 ----
            chain_ctx = tc.tile_pool(name="chains", bufs=1, space="PSUM")
            chainp = chain_ctx.__enter__()
            chAB = chainp.tile([P, 512], F32, tag="chAB", name="chAB")
            chCz = chainp.tile([P, 512], F32, tag="chCz", name="chCz")
            res_h = [chAB[:, 0:D], chAB[:, D:2 * D], chCz[:, 0:D]]
            res_z = [chCz[:, D:D + 2], chCz[:, D + 2:D + 4],
                     chCz[:, D + 4:D + 6]]
            NRES = 3

            def agg2(pb, ic, ch_h, ch_z, start_h, start_z, stop):
                lhs = pts[pb][:, :, P * ic:P * ic + P]
                nc.tensor.matmul(
                    ch_h, lhs, haug[:, 2 * pb:2 * pb + 2, 0:D],
                    start=start_h, stop=stop, perf_mode=DR,
                    skip_group_check=True)
                nc.tensor.matmul(
                    ch_z, lhs, haug[:, 2 * pb:2 * pb + 2, D:DA],
                    start=start_z, stop=stop, perf_mode=DR,
                    skip_group_check=True)

            def agg(pb, ic, chain, start, stop):
                agg2(pb, ic, chain[:, 0:D], chain[:, D:DA], start, False,
                     stop)

            with tc.tile_pool(name="eps", bufs=3, space="PSUM") as eps:
                for pb in range(NPAIR):
                    if pb == 2:
                        for g in (8, 9, 10, 11):
                            mask_dma(g, nc.sync)
                        nc.gpsimd.dma_start(haug[:, 48:64, :],
                                            haug_d[:, 48:64, :])
                        for g in (12, 13):
                            mask_dma(g, nc.sync)
                        nc.sync.dma_start(xlp[:], xloc_d[:])
                        nc.sync.dma_start(fpk[:], fpk_d[:])
                        for g in (14, 15):
                            mask_dma(g, nc.sync)
                    for k in range(2):
                        jb = 2 * pb + k
                        path = paths[jb]
                        mk = mask_tiles[jb // 4][:, jb % 4, :]
                        rbj = rb[:, jb:jb + 1]
                        ps = eps.tile([P, NL], F32, tag="e")
                        if path == 5:
                            for c2 in range(2):
                                csl = slice(512 * c2, 512 * c2 + 512)
                                nc.tensor.matmul(
                                    ps[:, csl], idt[:], mk[:, csl],
                                    start=True, stop=False,
                                    skip_group_check=True)
                        for c4 in range(4):
                            nc.tensor.matmul(
                                ps[:, 256 * c4:256 * c4 + 256],
                                xthi[:, :, P * jb:P * jb + P],
                                qtp[:, :, 256 * c4:256 * c4 + 256],
                                start=(path != 5 and c4 % 2 == 0),
                                stop=(c4 % 2 == 1),
                                perf_mode=DR, skip_group_check=True)
                        pslot = pts[pb][:, k, :]
                        if path == 5:
                            nc.scalar.activation(pslot, ps[:], AF.Exp,
                                                 bias=rbj, scale=1.0)
                        else:
                            esc = escp.tile([P, NL], F32, tag="esc")
                            nc.vector.scalar_tensor_tensor(
                                out=esc[:], in0=ps[:], scalar=rbj,
                                in1=mk, op0=ALU.add, op1=ALU.add)
                            nc.gpsimd.tensor_tensor(pslot, ebase[:], esc[:],
                                                    op=ALU.pow)
                    if pb > 0:
                        for ic in range(NRES):
                            agg2(pb - 1, ic, res_h[ic], res_z[ic],
                                 start_h=(pb - 1 == 0 and ic == 0),
                                 start_z=(pb - 1 == 0 and ic == 0),
                                 stop=False)
                for ic in range(NRES):
                    agg2(NPAIR - 1, ic, res_h[ic], res_z[ic],
                         start_h=False, start_z=False, stop=True)

            # gate x-half dots (DVE free after the drain window)
            for ic in range(ICH):
                scr = hp4.tile([P, D], F32, tag="scr")
                nc.vector.scalar_tensor_tensor(
                    out=scr[:], in0=xlp[:, ic, :], scalar=1.0, in1=gwx_b[:],
                    op0=ALU.mult, op1=ALU.mult, accum_out=sxs[ic][:])

            # ---- phase 4 + tail chains ----
            def phase4(ch_h, ch_z, ic, outq=nc.sync):
                zrec = smallp.tile([P, 1], F32, tag="zrec")
                nc.vector.reciprocal(zrec[:], ch_z)
                hp = hp4.tile([P, D], F32, tag="hp")
                nc.scalar.activation(hp[:], ch_h, AF.Relu,
                                     bias=0.0, scale=zrec[:])
                scr2 = hp4.tile([P, D], F32, tag="scr")
                sh = smallp.tile([P, 1], F32, tag="sh")
                nc.vector.scalar_tensor_tensor(
                    out=scr2[:], in0=hp[:], scalar=1.0, in1=gwh_b[:],
                    op0=ALU.mult, op1=ALU.mult, accum_out=sh[:])
                st = smallp.tile([P, 1], F32, tag="st")
                nc.gpsimd.tensor_tensor(st[:], sxs[ic][:], sh[:], op=ALU.add)
                cf = smallp.tile([P, 1], F32, tag="cf")
                nc.scalar.activation(cf[:], st[:], AF.Sigmoid,
                                     bias=gbt, scale=1.0)
                dif = hp4.tile([P, D], F32, tag="scr")
                nc.gpsimd.tensor_tensor(dif[:], xlp[:, ic, :], hp[:],
                                        op=ALU.subtract)
                nc.vector.scalar_tensor_tensor(
                    out=otp[:, ic, :], in0=dif[:], scalar=cf[:],
                    in1=hp[:], op0=ALU.mult, op1=ALU.add)
                outq.dma_start(out_d[:, ic, :], otp[:, ic, :])

            with tc.tile_pool(name="tailps", bufs=1, space="PSUM") as tailps:
                tails = [tailps.tile([P, DA], F32, tag=f"tc{i}", name=f"tc{i}")
                         for i in range(5)]
                for ic in range(NRES, ICH):
                    ct = tails[ic - NRES]
                    for pb in range(NPAIR):
                        agg(pb, ic, ct, start=(pb == 0),
                            stop=(pb == NPAIR - 1))
                    if ic - NRES < NRES:
                        j = ic - NRES
                        phase4(res_h[j], res_z[j][:, 0:1], j)
                for ic in range(NRES, ICH):
                    ct = tails[ic - NRES]
                    phase4(ct[:, 0:D], ct[:, D:D + 1], ic,
                           nc.scalar if ic >= 6 else nc.sync)
            chain_ctx.__exit__(None, None, None)

    nc.compile()
    return nc


def prepare(x, edge_index, W_w, W_b, A, gate_w, gate_b):
    x64 = np.asarray(x, dtype=np.float64)
    W = np.asarray(W_w, dtype=np.float64)
    b = np.asarray(W_b, dtype=np.float64)
    A64 = np.asarray(A, dtype=np.float64)
    gate_w = np.asarray(gate_w, dtype=np.float32)
    gb = float(np.asarray(gate_b).reshape(-1)[0])
    assert x64.shape == (N, D)

    paths = tuple(PATHS)
    key = paths
    if key not in _BUILD_CACHE:
        _BUILD_CACHE[key] = _build(paths)
    nc = _BUILD_CACHE[key]

    asym = A64 + A64.T
    Qf = W.T @ asym @ W
    c1 = W.T @ asym.T @ b
    c2 = W.T @ asym @ b
    c0 = float(b @ asym @ b)

    # x^T e4m3 pairs [d%128, d//128, n]
    xq8 = np.ascontiguousarray(
        x64.T.astype(F8NP).reshape(2, P, N).transpose(1, 0, 2))
    # haug rows [h | 1 | 0] e4m3, grouped [128, 64, 258]
    h = (x64 @ W.T + b).astype(F8NP)
    haug = np.zeros((P, JBLK, DA), F8NP)
    haug[:, :, D] = F8NP(1.0)
    haug[:, :, 0:D] = h.reshape(JBLK, P, D).transpose(1, 0, 2)
    haug = np.ascontiguousarray(haug)
    # per-j row bias (exact f32)
    rbv = (x64 @ c1 + c0 - SH).astype(np.float32)
    rb = np.ascontiguousarray(rbv.reshape(JBLK, P).T)
    # gate/bias pack
    fpk = np.zeros((P, 2 + 3 * D), np.float32)
    fpk[:, 0] = gb
    fpk[:, 2:2 + D] = gate_w[:, :D]
    fpk[:, 2 + D:2 + 2 * D] = gate_w[:, D:]
    fpk[:, 2 + 2 * D:2 + 3 * D] = b.astype(np.float32)[None, :]
    fpk = np.ascontiguousarray(fpk)
    idt = np.ascontiguousarray(np.eye(P, dtype=F8NP))

    # adjacency
    adj = np.zeros((N, N), dtype=bool)
    s = np.asarray(edge_index[0], dtype=np.int64)
    d = np.asarray(edge_index[1], dtype=np.int64)
    adj[s, d] = True
    idx = np.arange(N)
    adj[idx, idx] = True

    cx = x64 @ c2
    am_edge = cx.astype(F8NP)          # per-i value on edges
    am_non = (cx - BNEG).astype(F8NP)  # off edges

    in_maps = []
    for c in range(M):
        xl = x64[c * NL:(c + 1) * NL]
        qtp = np.ascontiguousarray(
            (Qf.T @ xl.T).astype(F8NP).reshape(2, P, NL).transpose(1, 0, 2))
        sl = adj[c * NL:(c + 1) * NL, :].T  # [N(j), NL(i)] bool
        ame = am_edge[c * NL:(c + 1) * NL]
        amn = am_non[c * NL:(c + 1) * NL]
        mt = np.where(sl, ame[None, :], amn[None, :])
        mp = mt.reshape(JBLK // 4, 4, P, NL).transpose(0, 2, 1, 3)
        in_maps.append(dict(
            xthi=xq8, qtp=qtp, haugd=haug, rb=rb, fpk=fpk, idt=idt,
            mask=np.ascontiguousarray(mp.reshape(JBLK // 4 * P, 4 * NL)),
            xloc=np.ascontiguousarray(
                xl.astype(np.float32).reshape(ICH, P, D).transpose(1, 0, 2)),
        ))
    return nc, in_maps


def kernel(x, edge_index, W_w, W_b, A, gate_w, gate_b):
    global LAST_RESULT
    nc, in_maps = prepare(x, edge_index, W_w, W_b, A, gate_w, gate_b)
    os.environ["BASS_NEVER_TRACE"] = "1"
    res = run_bass_kernel_spmd(nc, in_maps, core_ids=list(range(M)))
    LAST_RESULT = res
    out = np.concatenate(
        [res.results[c]["out"].transpose(1, 0, 2).reshape(NL, D)
         for c in range(M)], axis=0)
    return out
